# revision 22
# baseline (speedup 1.0000x reference)
"""Distributed Trainium2 (8 NeuronCores) attention kernel.

Reference computation (per batch b):
    q = rope(x @ wq.T), k = rope(x @ wk.T), v = x @ wv.T     (16 heads, hd=128)
    out = softmax(q k^T / sqrt(hd) + mask) v  @ wo.T

Sharding: core c handles batch b = c//4 and head-group g = c%4 (4 heads).
Per-core pipeline (all matmuls bf16 with fp32 PSUM accumulation):
  1. QT/KT = w.T-major projections straight into the transposed [j, s]
     layout the attention matmuls want; RoPE is applied with head-dims
     de-interleaved (host permutes wq/wk rows so rope pairs are
     (i, i+64) -> clean [64, 512] partition-block vector ops).
  2. Flash-style attention with transposed scores ST[k, q]:
     ST = KT_blk.T @ QT (contract over head dim), exp on ScalarE,
     PV as V_blk.T @ PT giving OT[j, q] directly (no transposes),
     softmax denominator via a ones-column matmul, normalization via
     reciprocal + ones-broadcast matmul + vector multiply.
  3. One 8-core AllToAll exchanges normalized OT so each core ends up
     with the full-head OT for its own 512-row sequence strip; rows of
     the other batch are neutralized by host-zeroed wo_big rows (keeps
     the SPMD graph core-independent).
  4. Local output projection -> out strip [512, 2048] fp32.

Host reassembles the 8 strips into the [2, 2048, 2048] output.
"""

import numpy as np
import ml_dtypes

import concourse.bass as bass
import concourse.bacc as bacc
import concourse.mybir as mybir
import concourse.tile as tile
from concourse.bass_utils import run_bass_kernel_spmd

BF16 = mybir.dt.bfloat16
F32 = mybir.dt.float32
NPBF16 = ml_dtypes.bfloat16

N_CORES = 8
B, S, D = 2, 2048, 2048
NH = 16            # total heads
HD = 128           # head dim
NHL = 4            # heads per core
JW = NHL * HD      # 512 local head width
NKT = D // 128     # 16 contraction tiles for projections
NQC = S // 512     # 4 sequence chunks of 512
NSB = S // 128     # 16 sequence blocks of 128
MASK_NEG = -60.0   # effective -inf for exp (scores are O(5))

_GRAPH_CACHE = {}


def build_graph(causal: bool):
    nc = bacc.Bacc("TRN2", target_bir_lowering=False, debug=False,
                   num_devices=N_CORES)

    # ---- per-core DRAM parameters -------------------------------------
    xT = nc.declare_dram_parameter("xT", [D, S], BF16, isOutput=False)
    wqT = nc.declare_dram_parameter("wqT", [D, JW], BF16, isOutput=False)
    wkT = nc.declare_dram_parameter("wkT", [D, JW], BF16, isOutput=False)
    wvT = nc.declare_dram_parameter("wvT", [D, JW], BF16, isOutput=False)
    wo_all = nc.declare_dram_parameter("wo_all", [D, D], BF16, isOutput=False)
    gidx = nc.declare_dram_parameter("gidx", [128, 16], mybir.dt.int32,
                                     isOutput=False)
    cos2 = nc.declare_dram_parameter("cos2", [HD, S], BF16, isOutput=False)
    sgn2 = nc.declare_dram_parameter("sgn2", [HD, S], BF16, isOutput=False)
    ones = nc.declare_dram_parameter("ones", [128, 128], BF16, isOutput=False)
    eye = nc.declare_dram_parameter("eye", [128, 128], BF16, isOutput=False)
    if causal:
        bnd = nc.declare_dram_parameter("bnd", [512, 512], BF16, isOutput=False)
    else:
        maskT = nc.declare_dram_parameter("maskT", [S, S], BF16, isOutput=False)
    out = nc.declare_dram_parameter("out", [512, D], F32, isOutput=True)

    EXP = mybir.ActivationFunctionType.Exp

    with tile.TileContext(nc) as tc:
        with (
            tc.tile_pool(name="persist", bufs=1) as persist,
            tc.tile_pool(name="stream", bufs=5) as stream,
            tc.tile_pool(name="scratch", bufs=2) as scratch,
            tc.tile_pool(name="ps_mm", bufs=3, space="PSUM") as ps_mm,
            tc.tile_pool(name="ps_ot", bufs=3, space="PSUM") as ps_ot,
            tc.tile_pool(name="ps_sum", bufs=2, space="PSUM") as ps_sum,
            tc.tile_pool(name="dram", bufs=1, space="DRAM") as dram,
        ):
            ph1_cm = tc.tile_pool(name="ph1", bufs=1)
            ph1 = ph1_cm.__enter__()
            # ---- resident input tiles --------------------------------
            # interleave weight/x DMAs so the first projection matmuls can
            # start as soon as the first (wq, xt) tile pair lands
            w_sb = {"q": [], "k": [], "v": []}
            xt_sb = []
            # spread input loads over three DMA queues (queue = trigger
            # engine), so the first-wave 14MB lands ~3x faster
            qeng = [nc.sync, nc.scalar, nc.sync]
            for i in range(NKT):
                for k_, (nm, h) in enumerate((("q", wqT), ("k", wkT),
                                              ("v", wvT))):
                    t = ph1.tile([128, JW], BF16, tag=f"w{nm}{i}",
                                 name=f"w{nm}{i}")
                    qeng[k_].dma_start(t[:], h[128 * i:128 * (i + 1), :])
                    w_sb[nm].append(t)
                t = ph1.tile([128, S], BF16, tag=f"xt{i}", name=f"xt{i}")
                qeng[i % 3].dma_start(t[:], xT[128 * i:128 * (i + 1), :])
                xt_sb.append(t)
            cos_sb = persist.tile([HD, S], BF16, tag="cos", name="cos")
            sgn_sb = persist.tile([HD, S], BF16, tag="sin", name="sin")
            nc.sync.dma_start(cos_sb[:], cos2[:, :])
            nc.sync.dma_start(sgn_sb[:], sgn2[:, :])
            ones_sb = persist.tile([128, 128], BF16, tag="ones", name="ones")
            nc.sync.dma_start(ones_sb[:], ones[:, :])
            eye_sb = persist.tile([128, 128], BF16, tag="eye", name="eye")
            nc.sync.dma_start(eye_sb[:], eye[:, :])
            gidx_sb = persist.tile([128, 16], mybir.dt.int32, tag="gidx",
                                   name="gidx")
            nc.sync.dma_start(gidx_sb[:], gidx[:, :])
            if causal:
                bnd_sb = []
                for i in range(4):
                    t = persist.tile([128, 512], BF16, tag=f"bnd{i}", name=f"bnd{i}")
                    nc.sync.dma_start(t[:], bnd[128 * i:128 * (i + 1), :])
                    bnd_sb.append(t)

            # attention working tensors (persist across phases)
            qt_sb = [persist.tile([128, S], BF16, tag=f"qt{h}", name=f"qt{h}")
                     for h in range(NHL)]
            kt_sb = [persist.tile([128, S], BF16, tag=f"kt{h}", name=f"kt{h}")
                     for h in range(NHL)]
            v_sb = [persist.tile([128, JW], BF16, tag=f"v{i}", name=f"v{i}")
                    for i in range(NSB)]

            cc_in = dram.tile([8 * JW, 512], BF16, tag="cc_in", name="cc_in")
            cc_out = dram.tile([8 * JW, 512], BF16, tag="cc_out",
                               name="cc_out")

            # ---- phase 1: QKV projections + rope ---------------------
            def rope_into(dst, psum, qc):
                """Rope with de-interleaved head dims (pairs at i, i+64):
                dst = [A;B]*cos2 + [B;A]*sgn2  where sgn2 = [-sin; +sin].

                All DVE operands partition-aligned bf16 SBUF (2x mode)."""
                sl = slice(512 * qc, 512 * (qc + 1))
                stg = scratch.tile([128, 512], BF16, tag="stg", name="stg")
                nc.scalar.copy(stg[:], psum[:])
                sw = scratch.tile([128, 512], BF16, tag="sw", name="sw")
                nc.vector.tensor_copy(sw[0:64, :], stg[64:128, :])
                nc.vector.tensor_copy(sw[64:128, :], stg[0:64, :])
                u = scratch.tile([128, 512], BF16, tag="u", name="u")
                v = scratch.tile([128, 512], BF16, tag="v", name="v")
                nc.vector.tensor_mul(u[:], stg[:], cos_sb[:, sl])
                nc.vector.tensor_mul(v[:], sw[:], sgn_sb[:, sl])
                nc.vector.tensor_add(dst[:, sl], u[:], v[:])

            for qc in range(NQC):
                for h in range(NHL):
                    for nm, dsts in (("k", kt_sb), ("q", qt_sb)):
                        ps = ps_mm.tile([128, 512], F32, tag="mm", name="mm")
                        for dt in range(NKT):
                            nc.tensor.matmul(
                                ps[:],
                                w_sb[nm][dt][:, 128 * h:128 * (h + 1)],
                                xt_sb[dt][:, 512 * qc:512 * (qc + 1)],
                                start=(dt == 0), stop=(dt == NKT - 1),
                            )
                        rope_into(dsts[h], ps, qc)
            for sb_i in range(NSB):
                ps = ps_mm.tile([128, 512], F32, tag="mm", name="mm")
                for dt in range(NKT):
                    nc.tensor.matmul(
                        ps[:],
                        xt_sb[dt][:, 128 * sb_i:128 * (sb_i + 1)],
                        w_sb["v"][dt][:],
                        start=(dt == 0), stop=(dt == NKT - 1),
                    )
                nc.scalar.copy(v_sb[sb_i][:], ps[:])

            ph1_cm.__exit__(None, None, None)

            # ---- phase 2: attention ----------------------------------
            for qc in range(NQC):
                if not causal:
                    mt_sb = []
                    for kb in range(NSB):
                        t = stream.tile([128, 512], BF16, tag=f"mt{kb}", name=f"mt{kb}")
                        nc.sync.dma_start(
                            t[:], maskT[128 * kb:128 * (kb + 1),
                                        512 * qc:512 * (qc + 1)])
                        mt_sb.append(t)
                kbs = range(4 * qc + 4) if causal else range(NSB)
                for h in range(NHL):
                    ot_ps = ps_ot.tile([128, 512], F32, tag="ot", name="ot")
                    sum_ps = ps_sum.tile([1, 512], F32, tag="sum", name="sum")
                    last = kbs[-1]

                    def emit_scores(kb):
                        st = ps_mm.tile([128, 512], F32, tag="mm", name="mm")
                        has_mask = (kb >= 4 * qc) if causal else True
                        nc.tensor.matmul(
                            st[:],
                            kt_sb[h][:, 128 * kb:128 * (kb + 1)],
                            qt_sb[h][:, 512 * qc:512 * (qc + 1)],
                            start=True, stop=not has_mask,
                        )
                        if has_mask:
                            m_rhs = (bnd_sb[kb - 4 * qc] if causal
                                     else mt_sb[kb])
                            nc.tensor.matmul(st[:], eye_sb[:], m_rhs[:],
                                             start=False, stop=True)
                        pt = stream.tile([128, 512], BF16, tag="pt", name="pt")
                        nc.scalar.activation(pt[:], st[:], EXP)
                        return pt

                    def emit_pv(kb, pt):
                        nc.tensor.matmul(
                            ot_ps[:],
                            v_sb[kb][:, 128 * h:128 * (h + 1)],
                            pt[:],
                            start=(kb == 0), stop=(kb == last),
                        )
                        nc.tensor.matmul(
                            sum_ps[:],
                            ones_sb[:, 0:1],
                            pt[:],
                            start=(kb == 0), stop=(kb == last),
                        )

                    # software pipeline: scores(kb+1) issued before pv(kb) so
                    # the in-order PE never waits on exp of the current block
                    prev = None
                    for kb in kbs:
                        pt = emit_scores(kb)
                        if prev is not None:
                            emit_pv(prev[0], prev[1])
                        prev = (kb, pt)
                    emit_pv(prev[0], prev[1])
                    # normalize: r = approx 1/sums, partition-broadcast on
                    # gpsimd, OTn = OT * R
                    r_sb = scratch.tile([1, 512], F32, tag="rsb", name="rsb")
                    nc.vector.reciprocal_approx_fast(r_sb[:], sum_ps[:])
                    rb_sb = scratch.tile([128, 512], F32, tag="rbs", name="rbs")
                    nc.gpsimd.partition_broadcast(rb_sb[:], r_sb[:])
                    otn = stream.tile([128, 512], BF16, tag="otn", name="otn")
                    nc.vector.tensor_mul(otn[:], ot_ps[:], rb_sb[:])
                    # ship to both twin shards (qc and qc+4)
                    for p in (qc, qc + 4):
                        nc.sync.dma_start(
                            cc_in[512 * p + 128 * h:512 * p + 128 * (h + 1), :],
                            otn[:])

            # ---- phase 3: output projection --------------------------
            wopool_cm = tc.tile_pool(name="wopool", bufs=56)
            wopool = wopool_cm.__enter__()
            wo_tiles = {}
            for mc in range(4):
                for jt in range(16):
                    t = wopool.tile([128, 512], BF16, tag="wo", name="wo")
                    (nc.sync if jt % 2 == 0 else nc.scalar).dma_start(
                        t[:], wo_all[128 * jt:128 * (jt + 1),
                                     512 * mc:512 * (mc + 1)])
                    wo_tiles[(mc, jt)] = t
            ph3_cm = tc.tile_pool(name="ph3", bufs=1)
            ph3 = ph3_cm.__enter__()
            # self-shard partial outproj: my own strip's OT sits in cc_in
            # (shard index 4b+g, host-supplied indices) before the A2A even
            # starts -- run its 64 matmuls under the collective.
            nc.gpsimd.collective_compute(
                "AllToAll",
                mybir.AluOpType.bypass,
                replica_groups=[list(range(N_CORES))],
                ins=[cc_in.opt()],
                outs=[cc_out.opt()],
            )
            ot_self = []
            for jt in range(4):
                t = ph3.tile([128, 512], BF16, tag=f"otself{jt}",
                             name=f"otself{jt}")
                nc.gpsimd.indirect_dma_start(
                    out=t[:],
                    out_offset=None,
                    in_=cc_in[:],
                    in_offset=bass.IndirectOffsetOnAxis(
                        ap=gidx_sb[:, jt:jt + 1], axis=0),
                )
                ot_self.append(t)
            self_out = {}
            for mc in range(4):
                for ss in range(4):
                    po = ps_mm.tile([128, 512], F32, tag="mm", name="mm")
                    for jt in range(4):
                        nc.tensor.matmul(
                            po[:],
                            ot_self[jt][:, 128 * ss:128 * (ss + 1)],
                            wo_tiles[(mc, jt)][:],
                            start=(jt == 0), stop=(jt == 3),
                        )
                    t = ph3.tile([128, 512], F32, tag=f"so{mc}{ss}",
                                 name=f"so{mc}{ss}")
                    nc.scalar.copy(t[:], po[:])
                    self_out[(mc, ss)] = t
            ot_rx = []
            for jt in range(12):
                t = ph3.tile([128, 512], BF16, tag=f"otr{jt}", name=f"otr{jt}")
                nc.gpsimd.indirect_dma_start(
                    out=t[:],
                    out_offset=None,
                    in_=cc_out[:],
                    in_offset=bass.IndirectOffsetOnAxis(
                        ap=gidx_sb[:, 4 + jt:5 + jt], axis=0),
                )
                ot_rx.append(t)
            for mc in range(4):
                for ss in range(4):
                    po = ps_mm.tile([128, 512], F32, tag="mm", name="mm")
                    for jt in range(12):
                        nc.tensor.matmul(
                            po[:],
                            ot_rx[jt][:, 128 * ss:128 * (ss + 1)],
                            wo_tiles[(mc, 4 + jt)][:],
                            start=(jt == 0), stop=(jt == 11),
                        )
                    os_sb = scratch.tile([128, 512], F32, tag="os", name="os")
                    nc.vector.tensor_add(os_sb[:], po[:],
                                         self_out[(mc, ss)][:])
                    nc.sync.dma_start(
                        out[128 * ss:128 * (ss + 1),
                            512 * mc:512 * (mc + 1)], os_sb[:])
            ph3_cm.__exit__(None, None, None)
            wopool_cm.__exit__(None, None, None)

    nc.compile()
    return nc


def _prep_inputs(x, freqs_cos, freqs_sin, mask, wq, wk, wv, wo, causal):
    perm = np.concatenate(
        [h * HD + np.r_[np.arange(0, HD, 2), np.arange(1, HD, 2)]
         for h in range(NHL)])
    cosT = np.ascontiguousarray(freqs_cos.T.astype(np.float32))  # [64, S]
    sinT = np.ascontiguousarray(freqs_sin.T.astype(np.float32))
    cos2 = np.concatenate([cosT, cosT], axis=0)           # [128, S]
    sgn2 = np.concatenate([-sinT, sinT], axis=0)          # [128, S]
    ones = np.ones((128, 128), dtype=NPBF16)
    eye = np.eye(128, dtype=np.float32).astype(NPBF16)
    if causal:
        ki = np.arange(128)[:, None]
        qi = np.arange(512)[None, :]
        bnd = np.concatenate(
            [np.where(qi >= ki + 128 * i, 0.0, MASK_NEG)[None]
             for i in range(4)], axis=0).astype(np.float32).reshape(512, 512).astype(NPBF16)
    else:
        maskT = np.ascontiguousarray(
            np.maximum(mask, MASK_NEG).T.astype(NPBF16))

    in_maps = []
    for c in range(N_CORES):
        b, g = c // 4, c % 4
        rows = slice(JW * g, JW * (g + 1))
        wq_c = wq[rows][perm] * (HD ** -0.5)
        wk_c = wk[rows][perm]
        wv_c = wv[rows]
        # gather indices: cols 0..3 -> my own shard rows in cc_in;
        # cols 4..15 -> other head-groups' shards in cc_out (my batch)
        r = np.arange(128)[:, None]
        self_cols = 512 * (4 * b + g) + 128 * np.arange(4)[None, :] + r
        others = [gp for gp in range(4) if gp != g]
        oth_cols = np.concatenate(
            [2048 * b + 512 * gp + 128 * np.arange(4) for gp in others]
        )[None, :] + r
        gidx_np = np.concatenate([self_cols, oth_cols], axis=1).astype(np.int32)
        perm_rows = np.concatenate(
            [np.arange(JW * g, JW * (g + 1))]
            + [np.arange(JW * gp, JW * (gp + 1)) for gp in others])
        wo_allT = np.ascontiguousarray(wo.T[perm_rows]).astype(NPBF16)
        m = {
            "xT": np.ascontiguousarray(x[b].T).astype(NPBF16),
            "wqT": np.ascontiguousarray(wq_c.T).astype(NPBF16),
            "wkT": np.ascontiguousarray(wk_c.T).astype(NPBF16),
            "wvT": np.ascontiguousarray(wv_c.T).astype(NPBF16),
            "wo_all": wo_allT,
            "gidx": gidx_np,
            "cos2": cos2.astype(NPBF16),
            "sgn2": sgn2.astype(NPBF16),
            "ones": ones,
            "eye": eye,
        }
        if causal:
            m["bnd"] = bnd
        else:
            m["maskT"] = maskT
        in_maps.append(m)
    return in_maps


def kernel(x, start_pos, freqs_cos, freqs_sin, mask, wq, wk, wv, wo):
    x = np.asarray(x, dtype=np.float32)
    mask = np.asarray(mask, dtype=np.float32)
    wq, wk, wv, wo = (np.asarray(w, dtype=np.float32) for w in (wq, wk, wv, wo))
    freqs_cos = np.asarray(freqs_cos, dtype=np.float32)
    freqs_sin = np.asarray(freqs_sin, dtype=np.float32)
    assert x.shape == (B, S, D) and mask.shape == (S, S)

    canonical = np.triu(np.full((S, S), float("-inf"), dtype=np.float32), k=1)
    causal = bool(np.array_equal(mask, canonical))

    if causal not in _GRAPH_CACHE:
        _GRAPH_CACHE[causal] = build_graph(causal)
    nc = _GRAPH_CACHE[causal]

    in_maps = _prep_inputs(x, freqs_cos, freqs_sin, mask, wq, wk, wv, wo,
                           causal)
    res = run_bass_kernel_spmd(nc, in_maps, core_ids=list(range(N_CORES)))
    out = np.empty((B, S, D), dtype=np.float32)
    for c in range(N_CORES):
        b, g = c // 4, c % 4
        out[b, JW * g:JW * (g + 1), :] = res.results[c]["out"]
    return out


# revision 23
# speedup vs baseline: 1.0327x; 1.0327x over previous
"""Distributed Trainium2 (8 NeuronCores) attention kernel.

Reference computation (per batch b):
    q = rope(x @ wq.T), k = rope(x @ wk.T), v = x @ wv.T     (16 heads, hd=128)
    out = softmax(q k^T / sqrt(hd) + mask) v  @ wo.T

Sharding: core c handles batch b = c//4 and head-group g = c%4 (4 heads).
Per-core pipeline (all matmuls bf16 with fp32 PSUM accumulation):
  1. QT/KT = w.T-major projections straight into the transposed [j, s]
     layout the attention matmuls want; RoPE is applied with head-dims
     de-interleaved (host permutes wq/wk rows so rope pairs are
     (i, i+64) -> clean [64, 512] partition-block vector ops).
  2. Flash-style attention with transposed scores ST[k, q]:
     ST = KT_blk.T @ QT (contract over head dim), exp on ScalarE,
     PV as V_blk.T @ PT giving OT[j, q] directly (no transposes),
     softmax denominator via a ones-column matmul, normalization via
     reciprocal + ones-broadcast matmul + vector multiply.
  3. One 8-core AllToAll exchanges normalized OT so each core ends up
     with the full-head OT for its own 512-row sequence strip; rows of
     the other batch are neutralized by host-zeroed wo_big rows (keeps
     the SPMD graph core-independent).
  4. Local output projection -> out strip [512, 2048] fp32.

Host reassembles the 8 strips into the [2, 2048, 2048] output.
"""

import numpy as np
import ml_dtypes

import concourse.bass as bass
import concourse.bacc as bacc
import concourse.mybir as mybir
import concourse.tile as tile
from concourse.bass_utils import run_bass_kernel_spmd

BF16 = mybir.dt.bfloat16
F32 = mybir.dt.float32
NPBF16 = ml_dtypes.bfloat16

N_CORES = 8
B, S, D = 2, 2048, 2048
NH = 16            # total heads
HD = 128           # head dim
NHL = 4            # heads per core
JW = NHL * HD      # 512 local head width
NKT = D // 128     # 16 contraction tiles for projections
NQC = S // 512     # 4 sequence chunks of 512
NSB = S // 128     # 16 sequence blocks of 128
MASK_NEG = -60.0   # effective -inf for exp (scores are O(5))

_GRAPH_CACHE = {}


def build_graph(causal: bool):
    nc = bacc.Bacc("TRN2", target_bir_lowering=False, debug=False,
                   num_devices=N_CORES)

    # ---- per-core DRAM parameters -------------------------------------
    xT = nc.declare_dram_parameter("xT", [D, S], BF16, isOutput=False)
    wqT = nc.declare_dram_parameter("wqT", [D, JW], BF16, isOutput=False)
    wkT = nc.declare_dram_parameter("wkT", [D, JW], BF16, isOutput=False)
    wvT = nc.declare_dram_parameter("wvT", [D, JW], BF16, isOutput=False)
    wo_all = nc.declare_dram_parameter("wo_all", [D, D], BF16, isOutput=False)
    gidx = nc.declare_dram_parameter("gidx", [128, 16], mybir.dt.int32,
                                     isOutput=False)
    cos2 = nc.declare_dram_parameter("cos2", [HD, S], BF16, isOutput=False)
    sgn2 = nc.declare_dram_parameter("sgn2", [HD, S], BF16, isOutput=False)
    ones = nc.declare_dram_parameter("ones", [128, 128], BF16, isOutput=False)
    eye = nc.declare_dram_parameter("eye", [128, 128], BF16, isOutput=False)
    if causal:
        bnd = nc.declare_dram_parameter("bnd", [512, 512], BF16, isOutput=False)
    else:
        maskT = nc.declare_dram_parameter("maskT", [S, S], BF16, isOutput=False)
    out = nc.declare_dram_parameter("out", [512, D], F32, isOutput=True)

    EXP = mybir.ActivationFunctionType.Exp

    with tile.TileContext(nc) as tc:
        with (
            tc.tile_pool(name="persist", bufs=1) as persist,
            tc.tile_pool(name="stream", bufs=5) as stream,
            tc.tile_pool(name="scratch", bufs=2) as scratch,
            tc.tile_pool(name="ps_mm", bufs=3, space="PSUM") as ps_mm,
            tc.tile_pool(name="ps_ot", bufs=3, space="PSUM") as ps_ot,
            tc.tile_pool(name="ps_sum", bufs=2, space="PSUM") as ps_sum,
            tc.tile_pool(name="dram", bufs=1, space="DRAM") as dram,
        ):
            ph1_cm = tc.tile_pool(name="ph1", bufs=1)
            ph1 = ph1_cm.__enter__()
            # ---- resident input tiles --------------------------------
            # interleave weight/x DMAs so the first projection matmuls can
            # start as soon as the first (wq, xt) tile pair lands
            w_sb = {"q": [], "k": [], "v": []}
            xt_sb = []
            # spread input loads over three DMA queues (queue = trigger
            # engine), so the first-wave 14MB lands ~3x faster
            qeng = [nc.sync, nc.scalar, nc.sync]
            for i in range(NKT):
                for k_, (nm, h) in enumerate((("q", wqT), ("k", wkT),
                                              ("v", wvT))):
                    t = ph1.tile([128, JW], BF16, tag=f"w{nm}{i}",
                                 name=f"w{nm}{i}")
                    qeng[k_].dma_start(t[:], h[128 * i:128 * (i + 1), :])
                    w_sb[nm].append(t)
                t = ph1.tile([128, S], BF16, tag=f"xt{i}", name=f"xt{i}")
                qeng[i % 3].dma_start(t[:], xT[128 * i:128 * (i + 1), :])
                xt_sb.append(t)
            cos_sb = persist.tile([HD, S], BF16, tag="cos", name="cos")
            sgn_sb = persist.tile([HD, S], BF16, tag="sin", name="sin")
            nc.sync.dma_start(cos_sb[:], cos2[:, :])
            nc.sync.dma_start(sgn_sb[:], sgn2[:, :])
            ones_sb = persist.tile([128, 128], BF16, tag="ones", name="ones")
            nc.sync.dma_start(ones_sb[:], ones[:, :])
            eye_sb = persist.tile([128, 128], BF16, tag="eye", name="eye")
            nc.sync.dma_start(eye_sb[:], eye[:, :])
            gidx_sb = persist.tile([128, 16], mybir.dt.int32, tag="gidx",
                                   name="gidx")
            nc.sync.dma_start(gidx_sb[:], gidx[:, :])
            if causal:
                bnd_sb = []
                for i in range(4):
                    t = persist.tile([128, 512], BF16, tag=f"bnd{i}", name=f"bnd{i}")
                    nc.sync.dma_start(t[:], bnd[128 * i:128 * (i + 1), :])
                    bnd_sb.append(t)

            # attention working tensors (persist across phases)
            qt_sb = [persist.tile([128, S], BF16, tag=f"qt{h}", name=f"qt{h}")
                     for h in range(NHL)]
            kt_sb = [persist.tile([128, S], BF16, tag=f"kt{h}", name=f"kt{h}")
                     for h in range(NHL)]
            v_sb = [persist.tile([128, JW], BF16, tag=f"v{i}", name=f"v{i}")
                    for i in range(NSB)]

            cc_in = dram.tile([8 * JW, 512], BF16, tag="cc_in", name="cc_in")
            cc_out = dram.tile([8 * JW, 512], BF16, tag="cc_out",
                               name="cc_out")

            # ---- phase 1: QKV projections + rope ---------------------
            def rope_into(dst, psum, qc):
                """Rope with de-interleaved head dims (pairs at i, i+64):
                dst = [A;B]*cos2 + [B;A]*sgn2  where sgn2 = [-sin; +sin].

                All DVE operands partition-aligned bf16 SBUF (2x mode)."""
                sl = slice(512 * qc, 512 * (qc + 1))
                stg = scratch.tile([128, 512], BF16, tag="stg", name="stg")
                nc.scalar.copy(stg[:], psum[:])
                sw = scratch.tile([128, 512], BF16, tag="sw", name="sw")
                nc.vector.tensor_copy(sw[0:64, :], stg[64:128, :])
                nc.vector.tensor_copy(sw[64:128, :], stg[0:64, :])
                u = scratch.tile([128, 512], BF16, tag="u", name="u")
                v = scratch.tile([128, 512], BF16, tag="v", name="v")
                nc.vector.tensor_mul(u[:], stg[:], cos_sb[:, sl])
                nc.vector.tensor_mul(v[:], sw[:], sgn_sb[:, sl])
                nc.vector.tensor_add(dst[:, sl], u[:], v[:])

            for qc in range(NQC):
                for h in range(NHL):
                    for nm, dsts in (("k", kt_sb), ("q", qt_sb)):
                        ps = ps_mm.tile([128, 512], F32, tag="mm", name="mm")
                        for dt in range(NKT):
                            nc.tensor.matmul(
                                ps[:],
                                w_sb[nm][dt][:, 128 * h:128 * (h + 1)],
                                xt_sb[dt][:, 512 * qc:512 * (qc + 1)],
                                start=(dt == 0), stop=(dt == NKT - 1),
                            )
                        rope_into(dsts[h], ps, qc)
            for sb_i in range(NSB):
                ps = ps_mm.tile([128, 512], F32, tag="mm", name="mm")
                for dt in range(NKT):
                    nc.tensor.matmul(
                        ps[:],
                        xt_sb[dt][:, 128 * sb_i:128 * (sb_i + 1)],
                        w_sb["v"][dt][:],
                        start=(dt == 0), stop=(dt == NKT - 1),
                    )
                nc.scalar.copy(v_sb[sb_i][:], ps[:])

            ph1_cm.__exit__(None, None, None)

            # ---- phase 2: attention ----------------------------------
            for qc in range(NQC):
                if not causal:
                    mt_sb = []
                    for kb in range(NSB):
                        t = stream.tile([128, 512], BF16, tag=f"mt{kb}", name=f"mt{kb}")
                        nc.sync.dma_start(
                            t[:], maskT[128 * kb:128 * (kb + 1),
                                        512 * qc:512 * (qc + 1)])
                        mt_sb.append(t)
                kbs = range(4 * qc + 4) if causal else range(NSB)
                for h in range(NHL):
                    ot_ps = ps_ot.tile([128, 512], F32, tag="ot", name="ot")
                    sum_ps = ps_sum.tile([1, 512], F32, tag="sum", name="sum")
                    last = kbs[-1]

                    def emit_scores(kb):
                        st = ps_mm.tile([128, 512], F32, tag="mm", name="mm")
                        has_mask = (kb >= 4 * qc) if causal else True
                        nc.tensor.matmul(
                            st[:],
                            kt_sb[h][:, 128 * kb:128 * (kb + 1)],
                            qt_sb[h][:, 512 * qc:512 * (qc + 1)],
                            start=True, stop=not has_mask,
                        )
                        if has_mask:
                            m_rhs = (bnd_sb[kb - 4 * qc] if causal
                                     else mt_sb[kb])
                            nc.tensor.matmul(st[:], eye_sb[:], m_rhs[:],
                                             start=False, stop=True)
                        pt = stream.tile([128, 512], BF16, tag="pt", name="pt")
                        nc.scalar.activation(pt[:], st[:], EXP)
                        return pt

                    def emit_pv(kb, pt):
                        nc.tensor.matmul(
                            ot_ps[:],
                            v_sb[kb][:, 128 * h:128 * (h + 1)],
                            pt[:],
                            start=(kb == 0), stop=(kb == last),
                        )
                        nc.tensor.matmul(
                            sum_ps[:],
                            ones_sb[:, 0:1],
                            pt[:],
                            start=(kb == 0), stop=(kb == last),
                        )

                    # software pipeline: scores(kb+1) issued before pv(kb) so
                    # the in-order PE never waits on exp of the current block
                    prev = None
                    for kb in kbs:
                        pt = emit_scores(kb)
                        if prev is not None:
                            emit_pv(prev[0], prev[1])
                        prev = (kb, pt)
                    emit_pv(prev[0], prev[1])
                    # normalize: r = approx 1/sums, partition-broadcast on
                    # gpsimd, OTn = OT * R
                    r_sb = scratch.tile([1, 512], F32, tag="rsb", name="rsb")
                    nc.vector.reciprocal_approx_fast(r_sb[:], sum_ps[:])
                    rb_sb = scratch.tile([128, 512], F32, tag="rbs", name="rbs")
                    nc.gpsimd.partition_broadcast(rb_sb[:], r_sb[:])
                    otn = stream.tile([128, 512], BF16, tag="otn", name="otn")
                    nc.vector.tensor_mul(otn[:], ot_ps[:], rb_sb[:])
                    # ship to both twin shards (qc and qc+4)
                    for p in (qc, qc + 4):
                        nc.sync.dma_start(
                            cc_in[512 * p + 128 * h:512 * p + 128 * (h + 1), :],
                            otn[:])

            # ---- phase 3: output projection --------------------------
            wopool_cm = tc.tile_pool(name="wopool", bufs=56)
            wopool = wopool_cm.__enter__()
            wo_tiles = {}
            for mc in range(4):
                for jt in range(16):
                    t = wopool.tile([128, 512], BF16, tag="wo", name="wo")
                    (nc.sync if jt % 2 == 0 else nc.scalar).dma_start(
                        t[:], wo_all[128 * jt:128 * (jt + 1),
                                     512 * mc:512 * (mc + 1)])
                    wo_tiles[(mc, jt)] = t
            ph3_cm = tc.tile_pool(name="ph3", bufs=1)
            ph3 = ph3_cm.__enter__()
            # self-shard partial outproj: my own strip's OT sits in cc_in
            # (shard index 4b+g, host-supplied indices) before the A2A even
            # starts -- run its 64 matmuls under the collective.
            ot_self = []
            for jt in range(4):
                t = ph3.tile([128, 512], BF16, tag=f"otself{jt}",
                             name=f"otself{jt}")
                nc.gpsimd.indirect_dma_start(
                    out=t[:],
                    out_offset=None,
                    in_=cc_in[:],
                    in_offset=bass.IndirectOffsetOnAxis(
                        ap=gidx_sb[:, jt:jt + 1], axis=0),
                )
                ot_self.append(t)
            nc.gpsimd.collective_compute(
                "AllToAll",
                mybir.AluOpType.bypass,
                replica_groups=[list(range(N_CORES))],
                ins=[cc_in.opt()],
                outs=[cc_out.opt()],
            )
            self_out = {}
            for mc in range(4):
                for ss in range(4):
                    po = ps_mm.tile([128, 512], F32, tag="mm", name="mm")
                    for jt in range(4):
                        nc.tensor.matmul(
                            po[:],
                            ot_self[jt][:, 128 * ss:128 * (ss + 1)],
                            wo_tiles[(mc, jt)][:],
                            start=(jt == 0), stop=(jt == 3),
                        )
                    t = ph3.tile([128, 512], F32, tag=f"so{mc}{ss}",
                                 name=f"so{mc}{ss}")
                    nc.scalar.copy(t[:], po[:])
                    self_out[(mc, ss)] = t
            ot_rx = []
            for jt in range(12):
                t = ph3.tile([128, 512], BF16, tag=f"otr{jt}", name=f"otr{jt}")
                nc.gpsimd.indirect_dma_start(
                    out=t[:],
                    out_offset=None,
                    in_=cc_out[:],
                    in_offset=bass.IndirectOffsetOnAxis(
                        ap=gidx_sb[:, 4 + jt:5 + jt], axis=0),
                )
                ot_rx.append(t)
            for mc in range(4):
                for ss in range(4):
                    po = ps_mm.tile([128, 512], F32, tag="mm", name="mm")
                    for jt in range(12):
                        nc.tensor.matmul(
                            po[:],
                            ot_rx[jt][:, 128 * ss:128 * (ss + 1)],
                            wo_tiles[(mc, 4 + jt)][:],
                            start=(jt == 0), stop=(jt == 11),
                        )
                    os_sb = scratch.tile([128, 512], F32, tag="os", name="os")
                    nc.vector.tensor_add(os_sb[:], po[:],
                                         self_out[(mc, ss)][:])
                    nc.sync.dma_start(
                        out[128 * ss:128 * (ss + 1),
                            512 * mc:512 * (mc + 1)], os_sb[:])
            ph3_cm.__exit__(None, None, None)
            wopool_cm.__exit__(None, None, None)

    nc.compile()
    return nc


def _prep_inputs(x, freqs_cos, freqs_sin, mask, wq, wk, wv, wo, causal):
    perm = np.concatenate(
        [h * HD + np.r_[np.arange(0, HD, 2), np.arange(1, HD, 2)]
         for h in range(NHL)])
    cosT = np.ascontiguousarray(freqs_cos.T.astype(np.float32))  # [64, S]
    sinT = np.ascontiguousarray(freqs_sin.T.astype(np.float32))
    cos2 = np.concatenate([cosT, cosT], axis=0)           # [128, S]
    sgn2 = np.concatenate([-sinT, sinT], axis=0)          # [128, S]
    ones = np.ones((128, 128), dtype=NPBF16)
    eye = np.eye(128, dtype=np.float32).astype(NPBF16)
    if causal:
        ki = np.arange(128)[:, None]
        qi = np.arange(512)[None, :]
        bnd = np.concatenate(
            [np.where(qi >= ki + 128 * i, 0.0, MASK_NEG)[None]
             for i in range(4)], axis=0).astype(np.float32).reshape(512, 512).astype(NPBF16)
    else:
        maskT = np.ascontiguousarray(
            np.maximum(mask, MASK_NEG).T.astype(NPBF16))

    in_maps = []
    for c in range(N_CORES):
        b, g = c // 4, c % 4
        rows = slice(JW * g, JW * (g + 1))
        wq_c = wq[rows][perm] * (HD ** -0.5)
        wk_c = wk[rows][perm]
        wv_c = wv[rows]
        # gather indices: cols 0..3 -> my own shard rows in cc_in;
        # cols 4..15 -> other head-groups' shards in cc_out (my batch)
        r = np.arange(128)[:, None]
        self_cols = 512 * (4 * b + g) + 128 * np.arange(4)[None, :] + r
        others = [gp for gp in range(4) if gp != g]
        oth_cols = np.concatenate(
            [2048 * b + 512 * gp + 128 * np.arange(4) for gp in others]
        )[None, :] + r
        gidx_np = np.concatenate([self_cols, oth_cols], axis=1).astype(np.int32)
        perm_rows = np.concatenate(
            [np.arange(JW * g, JW * (g + 1))]
            + [np.arange(JW * gp, JW * (gp + 1)) for gp in others])
        wo_allT = np.ascontiguousarray(wo.T[perm_rows]).astype(NPBF16)
        m = {
            "xT": np.ascontiguousarray(x[b].T).astype(NPBF16),
            "wqT": np.ascontiguousarray(wq_c.T).astype(NPBF16),
            "wkT": np.ascontiguousarray(wk_c.T).astype(NPBF16),
            "wvT": np.ascontiguousarray(wv_c.T).astype(NPBF16),
            "wo_all": wo_allT,
            "gidx": gidx_np,
            "cos2": cos2.astype(NPBF16),
            "sgn2": sgn2.astype(NPBF16),
            "ones": ones,
            "eye": eye,
        }
        if causal:
            m["bnd"] = bnd
        else:
            m["maskT"] = maskT
        in_maps.append(m)
    return in_maps


def kernel(x, start_pos, freqs_cos, freqs_sin, mask, wq, wk, wv, wo):
    x = np.asarray(x, dtype=np.float32)
    mask = np.asarray(mask, dtype=np.float32)
    wq, wk, wv, wo = (np.asarray(w, dtype=np.float32) for w in (wq, wk, wv, wo))
    freqs_cos = np.asarray(freqs_cos, dtype=np.float32)
    freqs_sin = np.asarray(freqs_sin, dtype=np.float32)
    assert x.shape == (B, S, D) and mask.shape == (S, S)

    canonical = np.triu(np.full((S, S), float("-inf"), dtype=np.float32), k=1)
    causal = bool(np.array_equal(mask, canonical))

    if causal not in _GRAPH_CACHE:
        _GRAPH_CACHE[causal] = build_graph(causal)
    nc = _GRAPH_CACHE[causal]

    in_maps = _prep_inputs(x, freqs_cos, freqs_sin, mask, wq, wk, wv, wo,
                           causal)
    res = run_bass_kernel_spmd(nc, in_maps, core_ids=list(range(N_CORES)))
    out = np.empty((B, S, D), dtype=np.float32)
    for c in range(N_CORES):
        b, g = c // 4, c % 4
        out[b, JW * g:JW * (g + 1), :] = res.results[c]["out"]
    return out


# revision 24
# speedup vs baseline: 1.0666x; 1.0329x over previous
"""Distributed Trainium2 (8 NeuronCores) attention kernel.

Reference computation (per batch b):
    q = rope(x @ wq.T), k = rope(x @ wk.T), v = x @ wv.T     (16 heads, hd=128)
    out = softmax(q k^T / sqrt(hd) + mask) v  @ wo.T

Sharding: core c handles batch b = c//4 and head-group g = c%4 (4 heads).
Per-core pipeline (all matmuls bf16 with fp32 PSUM accumulation):
  1. QT/KT = w.T-major projections straight into the transposed [j, s]
     layout the attention matmuls want; RoPE is applied with head-dims
     de-interleaved (host permutes wq/wk rows so rope pairs are
     (i, i+64) -> clean [64, 512] partition-block vector ops).
  2. Flash-style attention with transposed scores ST[k, q]:
     ST = KT_blk.T @ QT (contract over head dim), exp on ScalarE,
     PV as V_blk.T @ PT giving OT[j, q] directly (no transposes),
     softmax denominator via a ones-column matmul, normalization via
     reciprocal + ones-broadcast matmul + vector multiply.
  3. One 8-core AllToAll exchanges normalized OT so each core ends up
     with the full-head OT for its own 512-row sequence strip; rows of
     the other batch are neutralized by host-zeroed wo_big rows (keeps
     the SPMD graph core-independent).
  4. Local output projection -> out strip [512, 2048] fp32.

Host reassembles the 8 strips into the [2, 2048, 2048] output.
"""

import numpy as np
import ml_dtypes

import concourse.bass as bass
import concourse.bacc as bacc
import concourse.mybir as mybir
import concourse.tile as tile
from concourse.bass_utils import run_bass_kernel_spmd

BF16 = mybir.dt.bfloat16
F32 = mybir.dt.float32
NPBF16 = ml_dtypes.bfloat16

N_CORES = 8
B, S, D = 2, 2048, 2048
NH = 16            # total heads
HD = 128           # head dim
NHL = 4            # heads per core
JW = NHL * HD      # 512 local head width
NKT = D // 128     # 16 contraction tiles for projections
NQC = S // 512     # 4 sequence chunks of 512
NSB = S // 128     # 16 sequence blocks of 128
MASK_NEG = -60.0   # effective -inf for exp (scores are O(5))

_GRAPH_CACHE = {}


def build_graph(causal: bool):
    nc = bacc.Bacc("TRN2", target_bir_lowering=False, debug=False,
                   num_devices=N_CORES)

    # ---- per-core DRAM parameters -------------------------------------
    xT = nc.declare_dram_parameter("xT", [D, S], BF16, isOutput=False)
    wqT = nc.declare_dram_parameter("wqT", [D, JW], BF16, isOutput=False)
    wkT = nc.declare_dram_parameter("wkT", [D, JW], BF16, isOutput=False)
    wvT = nc.declare_dram_parameter("wvT", [D, JW], BF16, isOutput=False)
    wo_all = nc.declare_dram_parameter("wo_all", [D, D], BF16, isOutput=False)
    gidx = nc.declare_dram_parameter("gidx", [128, 16], mybir.dt.int32,
                                     isOutput=False)
    cos2 = nc.declare_dram_parameter("cos2", [HD, S], BF16, isOutput=False)
    sgn2 = nc.declare_dram_parameter("sgn2", [HD, S], BF16, isOutput=False)
    ones = nc.declare_dram_parameter("ones", [128, 128], BF16, isOutput=False)
    eye = nc.declare_dram_parameter("eye", [128, 128], BF16, isOutput=False)
    if causal:
        bnd = nc.declare_dram_parameter("bnd", [512, 512], BF16, isOutput=False)
    else:
        maskT = nc.declare_dram_parameter("maskT", [S, S], BF16, isOutput=False)
    out = nc.declare_dram_parameter("out", [512, D], F32, isOutput=True)

    EXP = mybir.ActivationFunctionType.Exp

    with tile.TileContext(nc) as tc:
        with (
            tc.tile_pool(name="persist", bufs=1) as persist,
            tc.tile_pool(name="stream", bufs=5) as stream,
            tc.tile_pool(name="scratch", bufs=2) as scratch,
            tc.tile_pool(name="ps_mm", bufs=3, space="PSUM") as ps_mm,
            tc.tile_pool(name="ps_ot", bufs=3, space="PSUM") as ps_ot,
            tc.tile_pool(name="ps_sum", bufs=2, space="PSUM") as ps_sum,
            tc.tile_pool(name="dram", bufs=1, space="DRAM") as dram,
        ):
            ph1_cm = tc.tile_pool(name="ph1", bufs=1)
            ph1 = ph1_cm.__enter__()
            # ---- resident input tiles --------------------------------
            # interleave weight/x DMAs so the first projection matmuls can
            # start as soon as the first (wq, xt) tile pair lands
            w_sb = {"q": [], "k": [], "v": []}
            xt_sb = []
            # spread input loads over three DMA queues (queue = trigger
            # engine), so the first-wave 14MB lands ~3x faster
            qeng = [nc.sync, nc.scalar, nc.sync]
            for i in range(NKT):
                for k_, (nm, h) in enumerate((("q", wqT), ("k", wkT),
                                              ("v", wvT))):
                    t = ph1.tile([128, JW], BF16, tag=f"w{nm}{i}",
                                 name=f"w{nm}{i}")
                    qeng[k_].dma_start(t[:], h[128 * i:128 * (i + 1), :])
                    w_sb[nm].append(t)
                t = ph1.tile([128, S], BF16, tag=f"xt{i}", name=f"xt{i}")
                qeng[i % 3].dma_start(t[:], xT[128 * i:128 * (i + 1), :])
                xt_sb.append(t)
            cos_sb = persist.tile([HD, S], BF16, tag="cos", name="cos")
            sgn_sb = persist.tile([HD, S], BF16, tag="sin", name="sin")
            nc.sync.dma_start(cos_sb[:], cos2[:, :])
            nc.sync.dma_start(sgn_sb[:], sgn2[:, :])
            ones_sb = persist.tile([128, 128], BF16, tag="ones", name="ones")
            nc.sync.dma_start(ones_sb[:], ones[:, :])
            eye_sb = persist.tile([128, 128], BF16, tag="eye", name="eye")
            nc.sync.dma_start(eye_sb[:], eye[:, :])
            gidx_sb = persist.tile([128, 16], mybir.dt.int32, tag="gidx",
                                   name="gidx")
            nc.sync.dma_start(gidx_sb[:], gidx[:, :])
            if causal:
                bnd_sb = []
                for i in range(4):
                    t = persist.tile([128, 512], BF16, tag=f"bnd{i}", name=f"bnd{i}")
                    nc.sync.dma_start(t[:], bnd[128 * i:128 * (i + 1), :])
                    bnd_sb.append(t)

            # attention working tensors (persist across phases)
            qt_sb = [persist.tile([128, S], BF16, tag=f"qt{h}", name=f"qt{h}")
                     for h in range(NHL)]
            kt_sb = [persist.tile([128, S], BF16, tag=f"kt{h}", name=f"kt{h}")
                     for h in range(NHL)]
            v_sb = [persist.tile([128, JW], BF16, tag=f"v{i}", name=f"v{i}")
                    for i in range(NSB)]

            cc_in = dram.tile([8 * JW, 512], BF16, tag="cc_in", name="cc_in")
            cc_out = dram.tile([8 * JW, 512], BF16, tag="cc_out",
                               name="cc_out")

            # ---- phase 1: QKV projections + rope ---------------------
            def rope_into(dst, psum, qc):
                """Rope with de-interleaved head dims (pairs at i, i+64):
                dst = [A;B]*cos2 + [B;A]*sgn2  where sgn2 = [-sin; +sin].

                All DVE operands partition-aligned bf16 SBUF (2x mode)."""
                sl = slice(512 * qc, 512 * (qc + 1))
                stg = scratch.tile([128, 512], BF16, tag="stg", name="stg")
                nc.scalar.copy(stg[:], psum[:])
                sw = scratch.tile([128, 512], BF16, tag="sw", name="sw")
                nc.vector.tensor_copy(sw[0:64, :], stg[64:128, :])
                nc.vector.tensor_copy(sw[64:128, :], stg[0:64, :])
                u = scratch.tile([128, 512], BF16, tag="u", name="u")
                v = scratch.tile([128, 512], BF16, tag="v", name="v")
                nc.vector.tensor_mul(u[:], stg[:], cos_sb[:, sl])
                nc.vector.tensor_mul(v[:], sw[:], sgn_sb[:, sl])
                nc.vector.tensor_add(dst[:, sl], u[:], v[:])

            def emit_proj_qk(qc):
                for h in range(NHL):
                    for nm, dsts in (("k", kt_sb), ("q", qt_sb)):
                        ps = ps_mm.tile([128, 512], F32, tag="mm", name="mm")
                        for dt in range(NKT):
                            nc.tensor.matmul(
                                ps[:],
                                w_sb[nm][dt][:, 128 * h:128 * (h + 1)],
                                xt_sb[dt][:, 512 * qc:512 * (qc + 1)],
                                start=(dt == 0), stop=(dt == NKT - 1),
                            )
                        rope_into(dsts[h], ps, qc)

            def emit_proj_v(sb_i):
                ps = ps_mm.tile([128, 512], F32, tag="mm", name="mm")
                for dt in range(NKT):
                    nc.tensor.matmul(
                        ps[:],
                        xt_sb[dt][:, 128 * sb_i:128 * (sb_i + 1)],
                        w_sb["v"][dt][:],
                        start=(dt == 0), stop=(dt == NKT - 1),
                    )
                nc.scalar.copy(v_sb[sb_i][:], ps[:])

            def emit_attention(qc, mt_sb):
                kbs = range(4 * qc + 4) if causal else range(NSB)
                qsl = slice(512 * qc, 512 * (qc + 1))
                for h in range(NHL):
                    ot_ps = ps_ot.tile([128, 512], F32, tag="ot", name="ot")
                    sum_ps = ps_sum.tile([1, 512], F32, tag="sum", name="sum")
                    last = kbs[-1]

                    def emit_scores(kb):
                        # within a diagonal block at offset i=kb-4qc, the
                        # first 128*i columns are fully masked: skip them
                        co = 128 * (kb - 4 * qc) if (causal and kb > 4 * qc) \
                            else 0
                        st = ps_mm.tile([128, 512], F32, tag="mm", name="mm")
                        has_mask = (kb >= 4 * qc) if causal else True
                        nc.tensor.matmul(
                            st[:, co:],
                            kt_sb[h][:, 128 * kb:128 * (kb + 1)],
                            qt_sb[h][:, 512 * qc + co:512 * (qc + 1)],
                            start=True, stop=not has_mask,
                        )
                        if has_mask:
                            if causal:
                                m_rhs = bnd_sb[kb - 4 * qc][:, co:]
                            else:
                                m_rhs = mt_sb[kb][:]
                            nc.tensor.matmul(st[:, co:], eye_sb[:], m_rhs,
                                             start=False, stop=True)
                        pt = stream.tile([128, 512], BF16, tag="pt",
                                         name="pt")
                        nc.scalar.activation(pt[:, co:], st[:, co:], EXP)
                        return pt, co

                    def emit_pv(kb, pt, co):
                        nc.tensor.matmul(
                            ot_ps[:, co:],
                            v_sb[kb][:, 128 * h:128 * (h + 1)],
                            pt[:, co:],
                            start=(kb == 0), stop=(kb == last),
                        )
                        nc.tensor.matmul(
                            sum_ps[:, co:],
                            ones_sb[:, 0:1],
                            pt[:, co:],
                            start=(kb == 0), stop=(kb == last),
                        )

                    # software pipeline: scores(kb+1) before pv(kb) so the
                    # in-order PE never waits on the current block's exp
                    prev = None
                    for kb in kbs:
                        pt, co = emit_scores(kb)
                        if prev is not None:
                            emit_pv(*prev)
                        prev = (kb, pt, co)
                    emit_pv(*prev)
                    # normalize: r = approx 1/sums, partition-broadcast on
                    # gpsimd, OTn = OT * R
                    r_sb = scratch.tile([1, 512], F32, tag="rsb", name="rsb")
                    nc.vector.reciprocal_approx_fast(r_sb[:], sum_ps[:])
                    rb_sb = scratch.tile([128, 512], F32, tag="rbs",
                                         name="rbs")
                    nc.gpsimd.partition_broadcast(rb_sb[:], r_sb[:])
                    otn = stream.tile([128, 512], BF16, tag="otn", name="otn")
                    nc.vector.tensor_mul(otn[:], ot_ps[:], rb_sb[:])
                    # ship to both twin shards (qc and qc+4)
                    for p in (qc, qc + 4):
                        nc.sync.dma_start(
                            cc_in[512 * p + 128 * h:512 * p + 128 * (h + 1),
                                  :],
                            otn[:])

            if causal:
                # fused pipeline: attention for chunk qc only needs K/V
                # through chunk qc, so the next chunk's projection matmuls
                # fill attention's exp-dependency bubbles on the PE
                for qc in range(NQC):
                    emit_proj_qk(qc)
                    for sb_i in range(4 * qc, 4 * qc + 4):
                        emit_proj_v(sb_i)
                    emit_attention(qc, None)
                ph1_cm.__exit__(None, None, None)
            else:
                for qc in range(NQC):
                    emit_proj_qk(qc)
                for sb_i in range(NSB):
                    emit_proj_v(sb_i)
                ph1_cm.__exit__(None, None, None)
                for qc in range(NQC):
                    mt_sb = []
                    for kb in range(NSB):
                        t = stream.tile([128, 512], BF16, tag=f"mt{kb}",
                                        name=f"mt{kb}")
                        nc.sync.dma_start(
                            t[:], maskT[128 * kb:128 * (kb + 1),
                                        512 * qc:512 * (qc + 1)])
                        mt_sb.append(t)
                    emit_attention(qc, mt_sb)

            # ---- phase 3: output projection --------------------------
            wopool_cm = tc.tile_pool(name="wopool", bufs=56)
            wopool = wopool_cm.__enter__()
            wo_tiles = {}
            for mc in range(4):
                for jt in range(16):
                    t = wopool.tile([128, 512], BF16, tag="wo", name="wo")
                    (nc.sync if jt % 2 == 0 else nc.scalar).dma_start(
                        t[:], wo_all[128 * jt:128 * (jt + 1),
                                     512 * mc:512 * (mc + 1)])
                    wo_tiles[(mc, jt)] = t
            ph3_cm = tc.tile_pool(name="ph3", bufs=1)
            ph3 = ph3_cm.__enter__()
            # self-shard partial outproj: my own strip's OT sits in cc_in
            # (shard index 4b+g, host-supplied indices) before the A2A even
            # starts -- run its 64 matmuls under the collective.
            ot_self = []
            for jt in range(4):
                t = ph3.tile([128, 512], BF16, tag=f"otself{jt}",
                             name=f"otself{jt}")
                nc.gpsimd.indirect_dma_start(
                    out=t[:],
                    out_offset=None,
                    in_=cc_in[:],
                    in_offset=bass.IndirectOffsetOnAxis(
                        ap=gidx_sb[:, jt:jt + 1], axis=0),
                )
                ot_self.append(t)
            nc.gpsimd.collective_compute(
                "AllToAll",
                mybir.AluOpType.bypass,
                replica_groups=[list(range(N_CORES))],
                ins=[cc_in.opt()],
                outs=[cc_out.opt()],
            )
            self_out = {}
            for mc in range(4):
                for ss in range(4):
                    po = ps_mm.tile([128, 512], F32, tag="mm", name="mm")
                    for jt in range(4):
                        nc.tensor.matmul(
                            po[:],
                            ot_self[jt][:, 128 * ss:128 * (ss + 1)],
                            wo_tiles[(mc, jt)][:],
                            start=(jt == 0), stop=(jt == 3),
                        )
                    t = ph3.tile([128, 512], F32, tag=f"so{mc}{ss}",
                                 name=f"so{mc}{ss}")
                    nc.scalar.copy(t[:], po[:])
                    self_out[(mc, ss)] = t
            ot_rx = []
            for jt in range(12):
                t = ph3.tile([128, 512], BF16, tag=f"otr{jt}", name=f"otr{jt}")
                nc.gpsimd.indirect_dma_start(
                    out=t[:],
                    out_offset=None,
                    in_=cc_out[:],
                    in_offset=bass.IndirectOffsetOnAxis(
                        ap=gidx_sb[:, 4 + jt:5 + jt], axis=0),
                )
                ot_rx.append(t)
            for mc in range(4):
                for ss in range(4):
                    po = ps_mm.tile([128, 512], F32, tag="mm", name="mm")
                    for jt in range(12):
                        nc.tensor.matmul(
                            po[:],
                            ot_rx[jt][:, 128 * ss:128 * (ss + 1)],
                            wo_tiles[(mc, 4 + jt)][:],
                            start=(jt == 0), stop=(jt == 11),
                        )
                    os_sb = scratch.tile([128, 512], F32, tag="os", name="os")
                    nc.vector.tensor_add(os_sb[:], po[:],
                                         self_out[(mc, ss)][:])
                    nc.sync.dma_start(
                        out[128 * ss:128 * (ss + 1),
                            512 * mc:512 * (mc + 1)], os_sb[:])
            ph3_cm.__exit__(None, None, None)
            wopool_cm.__exit__(None, None, None)

    nc.compile()
    return nc


def _prep_inputs(x, freqs_cos, freqs_sin, mask, wq, wk, wv, wo, causal):
    perm = np.concatenate(
        [h * HD + np.r_[np.arange(0, HD, 2), np.arange(1, HD, 2)]
         for h in range(NHL)])
    cosT = np.ascontiguousarray(freqs_cos.T.astype(np.float32))  # [64, S]
    sinT = np.ascontiguousarray(freqs_sin.T.astype(np.float32))
    cos2 = np.concatenate([cosT, cosT], axis=0)           # [128, S]
    sgn2 = np.concatenate([-sinT, sinT], axis=0)          # [128, S]
    ones = np.ones((128, 128), dtype=NPBF16)
    eye = np.eye(128, dtype=np.float32).astype(NPBF16)
    if causal:
        ki = np.arange(128)[:, None]
        qi = np.arange(512)[None, :]
        bnd = np.concatenate(
            [np.where(qi >= ki + 128 * i, 0.0, MASK_NEG)[None]
             for i in range(4)], axis=0).astype(np.float32).reshape(512, 512).astype(NPBF16)
    else:
        maskT = np.ascontiguousarray(
            np.maximum(mask, MASK_NEG).T.astype(NPBF16))

    in_maps = []
    for c in range(N_CORES):
        b, g = c // 4, c % 4
        rows = slice(JW * g, JW * (g + 1))
        wq_c = wq[rows][perm] * (HD ** -0.5)
        wk_c = wk[rows][perm]
        wv_c = wv[rows]
        # gather indices: cols 0..3 -> my own shard rows in cc_in;
        # cols 4..15 -> other head-groups' shards in cc_out (my batch)
        r = np.arange(128)[:, None]
        self_cols = 512 * (4 * b + g) + 128 * np.arange(4)[None, :] + r
        others = [gp for gp in range(4) if gp != g]
        oth_cols = np.concatenate(
            [2048 * b + 512 * gp + 128 * np.arange(4) for gp in others]
        )[None, :] + r
        gidx_np = np.concatenate([self_cols, oth_cols], axis=1).astype(np.int32)
        perm_rows = np.concatenate(
            [np.arange(JW * g, JW * (g + 1))]
            + [np.arange(JW * gp, JW * (gp + 1)) for gp in others])
        wo_allT = np.ascontiguousarray(wo.T[perm_rows]).astype(NPBF16)
        m = {
            "xT": np.ascontiguousarray(x[b].T).astype(NPBF16),
            "wqT": np.ascontiguousarray(wq_c.T).astype(NPBF16),
            "wkT": np.ascontiguousarray(wk_c.T).astype(NPBF16),
            "wvT": np.ascontiguousarray(wv_c.T).astype(NPBF16),
            "wo_all": wo_allT,
            "gidx": gidx_np,
            "cos2": cos2.astype(NPBF16),
            "sgn2": sgn2.astype(NPBF16),
            "ones": ones,
            "eye": eye,
        }
        if causal:
            m["bnd"] = bnd
        else:
            m["maskT"] = maskT
        in_maps.append(m)
    return in_maps


def kernel(x, start_pos, freqs_cos, freqs_sin, mask, wq, wk, wv, wo):
    x = np.asarray(x, dtype=np.float32)
    mask = np.asarray(mask, dtype=np.float32)
    wq, wk, wv, wo = (np.asarray(w, dtype=np.float32) for w in (wq, wk, wv, wo))
    freqs_cos = np.asarray(freqs_cos, dtype=np.float32)
    freqs_sin = np.asarray(freqs_sin, dtype=np.float32)
    assert x.shape == (B, S, D) and mask.shape == (S, S)

    canonical = np.triu(np.full((S, S), float("-inf"), dtype=np.float32), k=1)
    causal = bool(np.array_equal(mask, canonical))

    if causal not in _GRAPH_CACHE:
        _GRAPH_CACHE[causal] = build_graph(causal)
    nc = _GRAPH_CACHE[causal]

    in_maps = _prep_inputs(x, freqs_cos, freqs_sin, mask, wq, wk, wv, wo,
                           causal)
    res = run_bass_kernel_spmd(nc, in_maps, core_ids=list(range(N_CORES)))
    out = np.empty((B, S, D), dtype=np.float32)
    for c in range(N_CORES):
        b, g = c // 4, c % 4
        out[b, JW * g:JW * (g + 1), :] = res.results[c]["out"]
    return out
